# revision 13
# baseline (speedup 1.0000x reference)
"""BDH layer (sparse-attention GLA block) on 8 Trainium2 NeuronCores.

Sharding: data-parallel over B (2) x tensor-parallel over heads (4).
Core c handles batch c//4, head c%4. Each core computes its head's partial
decoder output yMLP; a 4-core AllReduce per batch group sums them and every
core finishes the final norms. Host gathers y from core 0 (b=0) / core 4 (b=1).

Self-contained: hardcodes the problem shapes (B=2, T=1024, D=256, NH=4,
N=4096), builds/compiles the Bass program once per process, and runs it via
run_bass_kernel_spmd on cores 0-7.
"""

import math
import numpy as np

import concourse.bass as bass
import concourse.mybir as mybir
import concourse.tile as tile
from concourse import bacc
from concourse.bass_utils import run_bass_kernel_spmd

F32 = mybir.dt.float32
F32R = mybir.dt.float32r
BF16 = mybir.dt.bfloat16
AF = mybir.ActivationFunctionType
ALU = mybir.AluOpType

# ---- problem constants ----
B, T, D, NH, N = 2, 1024, 256, 4, 4096
NK = N                      # per-head key width (one head per core)
C = 256                     # GLA chunk length used by this kernel (exact math)
ROPE_BASE = float(2 ** 18)
SCALE_BASE = 512.0
GATE_DIV = 1024.0
EPS = 1e-5
KT = NK // 128              # 32 k-tiles
NS = T // C                 # 4 sweeps
CC = C // 128               # 2
DT = D // 128               # 2
TT = T // 128               # 8
SCALE = N ** -0.5
N_CORES = 8
GROUPS = [[0, 1, 2, 3], [4, 5, 6, 7]]


def _rope_tables():
    d = 256
    inv_freq = 1.0 / (ROPE_BASE ** (np.arange(0, d, 2, dtype=np.float64) / d))
    t = np.arange(T, dtype=np.float64)
    freqs = t[:, None] * inv_freq[None, :]
    scale = (np.arange(0, d, 2, dtype=np.float64) + 0.4 * d) / (1.4 * d)
    power = (t - float(T // 2)) / SCALE_BASE
    sc = scale[None, :] ** power[:, None]
    cos = (np.cos(freqs) * sc).astype(np.float32)
    sin = (np.sin(freqs) * sc).astype(np.float32)
    return np.ascontiguousarray(cos.T), np.ascontiguousarray(sin.T)


def _build():
    nc = bacc.Bacc("TRN2", target_bir_lowering=False, debug=False,
                   num_devices=N_CORES)

    KS = KT // 4            # 4-ktile DMA slabs
    wenc = nc.dram_tensor("wenc", [KS, 128, 4 * DT * 128], F32R,
                          kind="ExternalInput")
    wgate = nc.dram_tensor("wgate", [KS, 128, 4 * DT * 128], F32R,
                           kind="ExternalInput")
    wencv = nc.dram_tensor("wencv", [KS, 128, 4 * DT * 128], F32R,
                           kind="ExternalInput")
    wdec = nc.dram_tensor("wdec", [KS, 128, 4 * D], F32R, kind="ExternalInput")
    xt = nc.dram_tensor("xt", [DT, 128, T], F32R, kind="ExternalInput")
    xv = nc.dram_tensor("xv", [TT, 128, D], F32R, kind="ExternalInput")
    cos_t = nc.dram_tensor("cos_t", [128, T], F32, kind="ExternalInput")
    sin_t = nc.dram_tensor("sin_t", [128, T], F32, kind="ExternalInput")
    triu = nc.dram_tensor("triu", [CC, 128, C], F32R, kind="ExternalInput")
    ident = nc.dram_tensor("ident", [128, 128], F32R, kind="ExternalInput")
    y_out = nc.dram_tensor("y", [TT, 128, D], F32, kind="ExternalOutput")

    ar_in = nc.dram_tensor("ar_in", [TT, 128, D], F32)
    ar_out = nc.dram_tensor("ar_out", [TT, 128, D], F32)

    ln_s = math.log(SCALE)
    relu_gate_scale = 1.0 / math.sqrt(GATE_DIV)

    with tile.TileContext(nc) as tc:
        with (
            tc.tile_pool(name="persist", bufs=1) as p_per,
            tc.tile_pool(name="wstream", bufs=2) as p_w,
            tc.tile_pool(name="wdecs", bufs=2) as p_wd,
            tc.tile_pool(name="tran", bufs=2) as p_t,
            tc.tile_pool(name="qk", bufs=6) as p_qk,
            tc.tile_pool(name="xsp", bufs=KT // 2 + 1) as p_xsp,
            tc.tile_pool(name="hpool", bufs=KT) as p_h,
            tc.tile_pool(name="small", bufs=2) as p_s,
            tc.tile_pool(name="ps_eg", bufs=2, space="PSUM") as ps_eg,
            tc.tile_pool(name="ps_at", bufs=1, space="PSUM") as ps_at,
            tc.tile_pool(name="ps_o", bufs=1, space="PSUM") as ps_o,
            tc.tile_pool(name="ps_upd", bufs=1, space="PSUM") as ps_upd,
        ):
            xt_sb = p_per.tile([128, DT * T], F32R, tag="xt")
            for d_ in range(DT):
                nc.sync.dma_start(xt_sb[:, d_ * T:(d_ + 1) * T], xt[d_])
            xv_sb = p_per.tile([128, TT * D], F32R, tag="xv")
            for t_ in range(TT):
                nc.sync.dma_start(xv_sb[:, t_ * D:(t_ + 1) * D], xv[t_])
            cos_sb = p_per.tile([128, T], F32, tag="cos")
            nc.sync.dma_start(cos_sb[:], cos_t[:])
            sin_sb = p_per.tile([128, T], F32, tag="sin")
            nc.sync.dma_start(sin_sb[:], sin_t[:])
            triu_sb = p_per.tile([128, CC * C], F32R, tag="triu")
            for j in range(CC):
                nc.sync.dma_start(triu_sb[:, j * C:(j + 1) * C], triu[j])
            id_sb = p_per.tile([128, 128], F32R, tag="ident")
            nc.sync.dma_start(id_sb[:], ident[:])
            id_f32 = id_sb[:].bitcast(F32)

            yMLP = p_per.tile([128, TT * D], F32, tag="ymlp")
            lns_c = p_per.tile([128, 1], F32, tag="lns")
            nc.gpsimd.memset(lns_c[:], ln_s)
            eps_c = p_per.tile([128, 1], F32, tag="epsc")
            nc.gpsimd.memset(eps_c[:], EPS)

            h_tiles = [p_h.tile([128, D], F32R, tag="h", name=f"h{i}")
                       for i in range(KT)]

            def xtile(d_, s):
                return xt_sb[:, d_ * T + s * C: d_ * T + (s + 1) * C]

            def vtile(s, j):
                t_ = s * CC + j
                return xv_sb[:, t_ * D:(t_ + 1) * D]

            for s in range(NS):
                csl = slice(s * C, (s + 1) * C)
                at_ps = [ps_at.tile([128, C], F32, tag=f"at{j}", name=f"at{j}_{s}")
                         for j in range(CC)]
                o_ps = [ps_o.tile([128, D], F32, tag=f"o{g}", name=f"o{g}_{s}")
                        for g in range(CC)]

                xsp_tiles = [None] * KT
                qg_tiles = [None] * KT
                kg_tiles = [None] * KT

                for grp in range(KT // 4):
                    kts = [4 * grp + u for u in range(4)]
                    wesl = p_w.tile([128, 4 * DT * 128], F32R, tag="wenc",
                                    name=f"wes{s}_{grp}")
                    nc.sync.dma_start(wesl[:], wenc[grp])
                    wgsl = p_w.tile([128, 4 * DT * 128], F32R, tag="wgate",
                                    name=f"wgs{s}_{grp}")
                    nc.sync.dma_start(wgsl[:], wgate[grp])
                    bneg = p_t.tile([128, 4 * C], F32, tag="bneg",
                                    name=f"bn{s}_{grp}", bufs=1)
                    xsp_pair = [None, None]
                    for half in range(2):   # kt pairs within the group
                        pe2 = ps_eg.tile([128, 2 * C], F32, tag="eg",
                                         name=f"pe{s}_{grp}_{half}")
                        pg2 = ps_eg.tile([128, 2 * C], F32, tag="eg",
                                         name=f"pg{s}_{grp}_{half}")
                        for u in range(2):
                            ki = 2 * half + u
                            for d_ in range(DT):
                                w_sl = wesl[:, ki * 256 + d_ * 128:
                                            ki * 256 + d_ * 128 + 128]
                                nc.tensor.matmul(pe2[:, u * C:(u + 1) * C],
                                                 w_sl, xtile(d_, s),
                                                 start=(d_ == 0),
                                                 stop=(d_ == DT - 1))
                            for d_ in range(DT):
                                w_sl = wgsl[:, ki * 256 + d_ * 128:
                                            ki * 256 + d_ * 128 + 128]
                                nc.tensor.matmul(pg2[:, u * C:(u + 1) * C],
                                                 w_sl, xtile(d_, s),
                                                 start=(d_ == 0),
                                                 stop=(d_ == DT - 1))
                        raw = p_t.tile([128, 2 * C], F32, tag="raw",
                                       name=f"raw{s}_{grp}_{half}", bufs=1)
                        nc.scalar.activation(raw[:], pe2[:], AF.Copy)
                        xsp = p_xsp.tile([128, 2 * C], F32, tag="xsp",
                                         name=f"xsp{s}_{grp}_{half}")
                        nc.vector.scalar_tensor_tensor(
                            xsp[:], pe2[:], 0.0, raw[:], ALU.max, ALU.mult)
                        xsp_pair[half] = xsp
                        kt0 = 4 * grp + 2 * half
                        xsp_tiles[kt0] = xsp[:, 0:C]
                        xsp_tiles[kt0 + 1] = xsp[:, C:2 * C]
                        rg = p_t.tile([128, 2 * C], F32, tag="rg",
                                      name=f"rg{s}_{grp}_{half}")
                        nc.scalar.activation(rg[:], pg2[:], AF.Relu,
                                             scale=relu_gate_scale)
                        g2 = p_t.tile([128, 2 * C], F32, tag="g2",
                                      name=f"g2{s}_{grp}_{half}")
                        nc.gpsimd.tensor_tensor(g2[:], rg[:], rg[:], ALU.mult)
                        for u in range(2):
                            nc.vector.tensor_tensor_scan(
                                bneg[:, (2 * half + u) * C:(2 * half + u + 1) * C],
                                g2[:, u * C:(u + 1) * C], g2[:, u * C:(u + 1) * C],
                                0.0, ALU.add, ALU.bypass)
                    eb = p_t.tile([128, 4 * C], F32R, tag="eb",
                                  name=f"eb{s}_{grp}", bufs=2)
                    nc.scalar.activation(eb[:], bneg[:], AF.Exp,
                                         scale=-1.0, bias=lns_c[:])
                    enb = p_t.tile([128, 4 * C], F32R, tag="enb",
                                   name=f"enb{s}_{grp}", bufs=2)
                    nc.scalar.activation(enb[:], bneg[:], AF.Exp)

                    cos_s, sin_s = cos_sb[:, csl], sin_sb[:, csl]
                    cos_b = cos_s.unsqueeze(1).broadcast_to([128, 2, C])
                    sin_b = sin_s.unsqueeze(1).broadcast_to([128, 2, C])
                    for half in range(2):
                        kt0 = 4 * grp + 2 * half
                        xsp = xsp_pair[half]
                        xsp3 = xsp[:].rearrange("p (a c) -> p a c", a=2)
                        mc = p_t.tile([128, 2 * C], F32, tag="mc",
                                      name=f"mc{s}_{grp}_{half}", bufs=1)
                        nc.vector.tensor_tensor(
                            mc[:].rearrange("p (a c) -> p a c", a=2),
                            xsp3, cos_b, ALU.mult)
                        ms_ = p_t.tile([128, 2 * C], F32, tag="ms_",
                                       name=f"msn{s}_{grp}_{half}", bufs=1)
                        nc.gpsimd.tensor_tensor(
                            ms_[:].rearrange("p (a c) -> p a c", a=2),
                            xsp3, sin_b, ALU.mult)
                        # mc = [x0*cos | x1*cos], ms_ = [x0*sin | x1*sin]
                        rot = p_qk.tile([128, 2 * C], F32, tag="rot",
                                        name=f"rot{s}_{grp}_{half}", bufs=2)
                        nc.vector.tensor_tensor(rot[:, 0:C], mc[:, 0:C],
                                                ms_[:, C:2 * C], ALU.subtract)
                        nc.vector.tensor_tensor(rot[:, C:2 * C], ms_[:, 0:C],
                                                mc[:, C:2 * C], ALU.add)
                        ebsl = eb[:, 2 * half * C:(2 * half + 2) * C]
                        enbsl = enb[:, 2 * half * C:(2 * half + 2) * C]
                        qg2 = p_qk.tile([128, 2 * C], F32R, tag="qg",
                                        name=f"qg{s}_{grp}_{half}", bufs=2)
                        nc.vector.tensor_tensor(qg2[:], rot[:], ebsl, ALU.mult)
                        kg2 = p_qk.tile([128, 2 * C], F32R, tag="kg",
                                        name=f"kg{s}_{grp}_{half}", bufs=2)
                        nc.gpsimd.tensor_tensor(kg2[:], rot[:], enbsl, ALU.mult)
                        qg_tiles[kt0] = qg2[:, 0:C]
                        qg_tiles[kt0 + 1] = qg2[:, C:2 * C]
                        kg_tiles[kt0] = kg2[:, 0:C]
                        kg_tiles[kt0 + 1] = kg2[:, C:2 * C]

                        # ---- GLA for this pair ----
                        if s < NS - 1:
                            tp = ps_upd.tile([128, 2 * C], F32, tag="tp",
                                             name=f"tp{s}_{grp}_{half}")
                        for u in range(2):
                            kt = kt0 + u
                            qg, kg = qg_tiles[kt], kg_tiles[kt]
                            first, last = (kt == 0), (kt == KT - 1)
                            for j in range(CC):
                                nc.tensor.matmul(at_ps[j][:],
                                                 kg[:, j * 128:(j + 1) * 128],
                                                 qg, start=first, stop=last)
                            if s > 0:
                                for g in range(CC):
                                    nc.tensor.matmul(
                                        o_ps[g][:], qg[:, g * 128:(g + 1) * 128],
                                        h_tiles[kt][:], start=first, stop=False)
                            if s < NS - 1:
                                for j in range(CC):
                                    nc.tensor.transpose(
                                        tp[:, (2 * u + j) * 128:
                                           (2 * u + j + 1) * 128],
                                        kg[:, j * 128:(j + 1) * 128].bitcast(F32),
                                        id_f32)
                        if s < NS - 1:
                            kgdbT = p_t.tile([128, 2 * C], F32R, tag="kgdbT",
                                             name=f"kgT{s}_{grp}_{half}")
                            nc.scalar.activation(kgdbT[:], tp[:], AF.Copy)
                            dbv = p_s.tile([128, 2], F32, tag="dbv",
                                           name=f"dbv{s}_{grp}_{half}", bufs=4)
                            nc.vector.tensor_scalar_mul(
                                dbv[:], ebsl.bitcast(F32)[:, C - 1:2 * C:C],
                                1.0 / SCALE)
                            for u in range(2):
                                kt = kt0 + u
                                upd = ps_upd.tile([128, D], F32, tag="upd",
                                                  name=f"up{s}_{kt}")
                                for j in range(CC):
                                    nc.tensor.matmul(
                                        upd[:],
                                        kgdbT[:, (2 * u + j) * 128:
                                              (2 * u + j + 1) * 128],
                                        vtile(s, j), start=(j == 0),
                                        stop=(s == 0 and j == CC - 1))
                                if s > 0:
                                    nc.tensor.matmul(upd[:], id_sb[:],
                                                     h_tiles[kt][:],
                                                     start=False, stop=True)
                                nc.scalar.activation(h_tiles[kt][:], upd[:],
                                                     AF.Copy,
                                                     scale=dbv[:, u:u + 1])

                at_sb = []
                for j in range(CC):
                    m = p_t.tile([128, C], F32R, tag=f"atsb{j}",
                                 name=f"atsb{j}_{s}")
                    nc.vector.tensor_tensor(
                        m[:], at_ps[j][:],
                        triu_sb[:, j * C:(j + 1) * C].bitcast(F32), ALU.mult)
                    at_sb.append(m)
                for g in range(CC):
                    for j in range(CC):
                        nc.tensor.matmul(o_ps[g][:],
                                         at_sb[j][:, g * 128:(g + 1) * 128],
                                         vtile(s, j),
                                         start=(s == 0 and j == 0),
                                         stop=(j == CC - 1))

                ykvt = p_s.tile([128, CC * C], F32R, tag="ykvt", name=f"ykvt{s}", bufs=1)
                for g in range(CC):
                    o_t = o_ps[g]
                    s1 = p_s.tile([128, 1], F32, tag="s1", name=f"s1_{s}_{g}")
                    nc.vector.tensor_reduce(s1[:], o_t[:], mybir.AxisListType.X,
                                            ALU.add)
                    sq = p_t.tile([128, D], F32, tag="sqscr", name=f"sq{s}_{g}")
                    s2 = p_s.tile([128, 1], F32, tag="s2", name=f"s2_{s}_{g}")
                    nc.scalar.activation(sq[:], o_t[:], AF.Square, accum_out=s2[:])
                    mean = p_s.tile([128, 1], F32, tag="mean", name=f"mn{s}_{g}")
                    nc.vector.tensor_scalar_mul(mean[:], s1[:], 1.0 / D)
                    var = p_s.tile([128, 1], F32, tag="var", name=f"vr{s}_{g}")
                    nc.vector.tensor_scalar_mul(var[:], s2[:], 1.0 / D)
                    msq = p_s.tile([128, 1], F32, tag="msq", name=f"mq{s}_{g}")
                    nc.vector.tensor_tensor(msq[:], mean[:], mean[:], ALU.mult)
                    nc.vector.tensor_tensor(var[:], var[:], msq[:], ALU.subtract)
                    std = p_s.tile([128, 1], F32, tag="std", name=f"sd{s}_{g}")
                    nc.scalar.activation(std[:], var[:], AF.Sqrt, bias=eps_c[:])
                    rstd = p_s.tile([128, 1], F32, tag="rstd", name=f"rs{s}_{g}")
                    nc.vector.reciprocal(rstd[:], std[:])
                    nbias = p_s.tile([128, 1], F32, tag="nbias", name=f"nb{s}_{g}")
                    nc.vector.tensor_tensor(nbias[:], mean[:], rstd[:], ALU.mult)
                    nc.vector.tensor_scalar_mul(nbias[:], nbias[:], -1.0)
                    ykv = p_t.tile([128, D], F32, tag="ykv", name=f"ykv{s}_{g}")
                    nc.scalar.activation(ykv[:], o_t[:], AF.Identity,
                                         scale=rstd[:], bias=nbias[:])
                    tp2 = ps_upd.tile([128, C], F32, tag="tp", name=f"tpy{s}_{g}")
                    for d_ in range(DT):
                        nc.tensor.transpose(tp2[:, d_ * 128:(d_ + 1) * 128],
                                            ykv[:, d_ * 128:(d_ + 1) * 128], id_f32)
                    for d_ in range(DT):
                        nc.scalar.activation(
                            ykvt[:, d_ * C + g * 128: d_ * C + (g + 1) * 128],
                            tp2[:, d_ * 128:(d_ + 1) * 128], AF.Copy)

                dec_ps = [ps_o.tile([128, D], F32, tag=f"o{g}", name=f"dc{g}_{s}")
                          for g in range(CC)]
                for kp in range(KT // 2):     # kt pairs
                    kt0 = 2 * kp
                    if kt0 % 4 == 0:
                        ks = kt0 // 4
                        wvsl = p_w.tile([128, 4 * DT * 128], F32R, tag="wencv",
                                        name=f"wvs{s}_{ks}")
                        nc.sync.dma_start(wvsl[:], wencv[ks])
                        wdsl = p_wd.tile([128, 4 * D], F32R, tag="wdec",
                                         name=f"wds{s}_{ks}")
                        nc.sync.dma_start(wdsl[:], wdec[ks])
                    evps = ps_eg.tile([128, 2 * C], F32, tag="eg",
                                      name=f"pv{s}_{kp}")
                    for u in range(2):
                        ki = (kt0 + u) % 4
                        for d_ in range(DT):
                            nc.tensor.matmul(
                                evps[:, u * C:(u + 1) * C],
                                wvsl[:, ki * 256 + d_ * 128:
                                     ki * 256 + d_ * 128 + 128],
                                ykvt[:, d_ * C:(d_ + 1) * C],
                                start=(d_ == 0), stop=(d_ == DT - 1))
                    t1 = p_t.tile([128, 2 * C], F32, tag="t1",
                                  name=f"t1_{s}_{kp}", bufs=1)
                    nc.vector.scalar_tensor_tensor(
                        t1[:, 0:C], evps[:, 0:C], 0.0, xsp_tiles[kt0],
                        ALU.max, ALU.mult)
                    nc.vector.scalar_tensor_tensor(
                        t1[:, C:2 * C], evps[:, C:2 * C], 0.0,
                        xsp_tiles[kt0 + 1], ALU.max, ALU.mult)
                    xy = p_t.tile([128, 2 * C], F32R, tag="xy",
                                  name=f"xy{s}_{kp}")
                    nc.vector.scalar_tensor_tensor(
                        xy[:], evps[:], 0.0, t1[:], ALU.bypass, ALU.mult)
                    for u in range(2):
                        kt = kt0 + u
                        ki = kt % 4
                        for g in range(CC):
                            nc.tensor.matmul(
                                dec_ps[g][:],
                                xy[:, u * C + g * 128: u * C + (g + 1) * 128],
                                wdsl[:, ki * D:(ki + 1) * D],
                                start=(kt == 0), stop=(kt == KT - 1))
                for g in range(CC):
                    t_ = s * CC + g
                    nc.scalar.activation(yMLP[:, t_ * D:(t_ + 1) * D],
                                         dec_ps[g][:], AF.Copy)

            if N_CORES > 1:
                for t_ in range(TT):
                    nc.sync.dma_start(ar_in[t_], yMLP[:, t_ * D:(t_ + 1) * D])
                nc.gpsimd.collective_compute(
                    "AllReduce", ALU.add, replica_groups=GROUPS,
                    ins=[ar_in[:]], outs=[ar_out[:]])
                for t_ in range(TT):
                    nc.sync.dma_start(yMLP[:, t_ * D:(t_ + 1) * D], ar_out[t_])
            ym2 = yMLP

            for t_ in range(TT):
                ym = ym2[:, t_ * D:(t_ + 1) * D]
                s1 = p_s.tile([128, 1], F32, tag="s1", name=f"fs1_{t_}")
                nc.vector.tensor_reduce(s1[:], ym, mybir.AxisListType.X, ALU.add)
                sq = p_t.tile([128, D], F32, tag="sqscr", name=f"fsq{t_}")
                s2 = p_s.tile([128, 1], F32, tag="s2", name=f"fs2_{t_}")
                nc.scalar.activation(sq[:], ym, AF.Square, accum_out=s2[:])
                mean = p_s.tile([128, 1], F32, tag="mean", name=f"fmn{t_}")
                nc.vector.tensor_scalar_mul(mean[:], s1[:], 1.0 / D)
                var = p_s.tile([128, 1], F32, tag="var", name=f"fvr{t_}")
                nc.vector.tensor_scalar_mul(var[:], s2[:], 1.0 / D)
                msq = p_s.tile([128, 1], F32, tag="msq", name=f"fmq{t_}")
                nc.vector.tensor_tensor(msq[:], mean[:], mean[:], ALU.mult)
                nc.vector.tensor_tensor(var[:], var[:], msq[:], ALU.subtract)
                std = p_s.tile([128, 1], F32, tag="std", name=f"fsd{t_}")
                nc.scalar.activation(std[:], var[:], AF.Sqrt, bias=eps_c[:])
                rstd = p_s.tile([128, 1], F32, tag="rstd", name=f"frs{t_}")
                nc.vector.reciprocal(rstd[:], std[:])
                nbias = p_s.tile([128, 1], F32, tag="nbias", name=f"fnb{t_}")
                nc.vector.tensor_tensor(nbias[:], mean[:], rstd[:], ALU.mult)
                nc.vector.tensor_scalar_mul(nbias[:], nbias[:], -1.0)
                ln = p_t.tile([128, D], F32, tag="ln", name=f"fln{t_}")
                nc.scalar.activation(ln[:], ym, AF.Identity,
                                     scale=rstd[:], bias=nbias[:])
                z = p_t.tile([128, D], F32, tag="z", name=f"fz{t_}")
                nc.vector.tensor_tensor(
                    z[:], ln[:], xv_sb[:, t_ * D:(t_ + 1) * D].bitcast(F32),
                    ALU.add)
                sq2 = p_t.tile([128, D], F32, tag="sqscr2", name=f"fq2{t_}")
                ms = p_s.tile([128, 1], F32, tag="ms", name=f"fms{t_}")
                nc.scalar.activation(sq2[:], z[:], AF.Square, accum_out=ms[:])
                nc.vector.tensor_scalar_mul(ms[:], ms[:], 1.0 / D)
                rms = p_s.tile([128, 1], F32, tag="rms", name=f"frm{t_}")
                nc.scalar.activation(rms[:], ms[:], AF.Sqrt, bias=eps_c[:])
                rr = p_s.tile([128, 1], F32, tag="rr", name=f"frr{t_}")
                nc.vector.reciprocal(rr[:], rms[:])
                yo = p_t.tile([128, D], F32, tag="yo", name=f"fy{t_}")
                nc.scalar.activation(yo[:], z[:], AF.Copy, scale=rr[:])
                nc.sync.dma_start(y_out[t_], yo[:])

    nc.compile()
    return nc


def _tile_w(W):
    # (D, NK) -> (KT//4, 128, 4*DT*128): slab ks holds k-tiles 4ks..4ks+3,
    # column order (ki, d, c) matching the SBUF slab layout.
    W = np.asarray(W, dtype=np.float32)
    a = W.reshape(DT, 128, KT // 4, 4, 128).transpose(2, 1, 3, 0, 4)
    return np.ascontiguousarray(a.reshape(KT // 4, 128, 4 * DT * 128))


def _tile_wdec(W):
    # (NK, D) -> (KT//4, 128, 4*D)
    W = np.asarray(W, dtype=np.float32)
    a = W.reshape(KT // 4, 4, 128, D).transpose(0, 2, 1, 3)
    return np.ascontiguousarray(a.reshape(KT // 4, 128, 4 * D))


_STATE = {}


def _get_nc():
    if "nc" not in _STATE:
        _STATE["nc"] = _build()
    return _STATE["nc"]


def _core_in_map(x_b, W_enc_h, W_gate_h, W_encv_h, W_dec_h, consts):
    cos_t, sin_t, triu, ident = consts
    xT = np.ascontiguousarray(x_b.T)
    return {
        "wenc": _tile_w(W_enc_h),
        "wgate": _tile_w(W_gate_h),
        "wencv": _tile_w(W_encv_h),
        "wdec": _tile_wdec(W_dec_h),
        "xt": np.ascontiguousarray(xT.reshape(DT, 128, T)),
        "xv": np.ascontiguousarray(x_b.reshape(TT, 128, D)),
        "cos_t": cos_t, "sin_t": sin_t, "triu": triu, "ident": ident,
    }


def make_in_maps(x, W_enc, W_gate, W_dec, W_encv):
    x = np.asarray(x, dtype=np.float32)
    W_enc = np.asarray(W_enc, dtype=np.float32)
    W_gate = np.asarray(W_gate, dtype=np.float32)
    W_dec = np.asarray(W_dec, dtype=np.float32)
    W_encv = np.asarray(W_encv, dtype=np.float32)

    cos_t, sin_t = _rope_tables()
    triu = np.zeros((CC, 128, C), dtype=np.float32)
    for j in range(CC):
        for p in range(128):
            triu[j, p, j * 128 + p:] = 1.0
    ident = np.eye(128, dtype=np.float32)
    consts = (cos_t, sin_t, triu, ident)

    in_maps = []
    for c in range(N_CORES):
        b, h = c // 4, c % 4
        nsl = slice(h * N, (h + 1) * N)
        in_maps.append(_core_in_map(
            x[b], W_enc[:, nsl], W_gate[:, nsl], W_encv[h], W_dec[nsl, :],
            consts))
    return in_maps


def _get_runner():
    """Cached jitted SPMD executable mirroring bass2jax.run_bass_via_pjrt, so
    repeated kernel() calls skip re-tracing/recompiling."""
    if "runner" in _STATE:
        return _STATE["runner"]
    import jax
    import concourse.mybir as mb
    from concourse import bass2jax as b2j
    from jax.experimental.shard_map import shard_map
    from jax.sharding import Mesh, PartitionSpec

    nc = _get_nc()
    b2j.install_neuronx_cc_hook()
    partition_name = (nc.partition_id_tensor.name
                      if nc.partition_id_tensor else None)
    in_names, out_names, out_avals, zero_outs = [], [], [], []
    for alloc in nc.m.functions[0].allocations:
        if not isinstance(alloc, mb.MemoryLocationSet):
            continue
        name = alloc.memorylocations[0].name
        if alloc.kind == "ExternalInput":
            if name != partition_name:
                in_names.append(name)
        elif alloc.kind == "ExternalOutput":
            shape = tuple(alloc.tensor_shape)
            dtype = mb.dt.np(alloc.dtype)
            out_names.append(name)
            out_avals.append(jax.core.ShapedArray(shape, dtype))
            zero_outs.append(np.zeros(shape, dtype))
    n_params = len(in_names)
    all_names = in_names + out_names
    if partition_name is not None:
        all_names = all_names + [partition_name]
    donate = tuple(range(n_params, n_params + len(out_names)))

    def _body(*args):
        operands = list(args)
        if partition_name is not None:
            operands.append(b2j.partition_id_tensor())
        return tuple(b2j._bass_exec_p.bind(
            *operands,
            out_avals=tuple(out_avals),
            in_names=tuple(all_names),
            out_names=tuple(out_names),
            lowering_input_output_aliases=(),
            sim_require_finite=True,
            sim_require_nnan=True,
            nc=nc,
        ))

    devices = jax.devices()[:N_CORES]
    mesh = Mesh(np.asarray(devices), ("core",))
    in_specs = (PartitionSpec("core"),) * (n_params + len(out_names))
    out_specs = (PartitionSpec("core"),) * len(out_names)
    sharded = jax.jit(
        shard_map(_body, mesh=mesh, in_specs=in_specs, out_specs=out_specs,
                  check_rep=False),
        donate_argnums=donate, keep_unused=True)
    _STATE["runner"] = (sharded, in_names, out_names, out_avals, zero_outs, mesh)
    return _STATE["runner"]


def _concat_inputs(in_maps, in_names):
    return [np.concatenate([np.asarray(in_maps[c][nm]) for c in range(N_CORES)],
                           axis=0) for nm in in_names]


def _run(in_maps):
    sharded, in_names, out_names, out_avals, zero_outs, mesh = _get_runner()
    concat_in = _concat_inputs(in_maps, in_names)
    concat_zeros = [np.zeros((N_CORES * z.shape[0], *z.shape[1:]), z.dtype)
                    for z in zero_outs]
    out_arrs = sharded(*concat_in, *concat_zeros)
    return {name: np.asarray(out_arrs[i]).reshape(N_CORES, *out_avals[i].shape)
            for i, name in enumerate(out_names)}


def kernel(x, W_enc, W_gate, W_dec, W_encv):
    in_maps = make_in_maps(x, W_enc, W_gate, W_dec, W_encv)
    outs = _run(in_maps)
    y0 = outs["y"][0].reshape(T, D)
    y1 = outs["y"][4].reshape(T, D)
    return np.stack([y0, y1]).astype(np.float32)


def time_device_exec(np_inputs, iters=10):
    """Best wall-clock (ns) of the device execution with inputs pre-staged on
    device; excludes host prep and output conversion."""
    import time as _time
    import jax
    from jax.sharding import NamedSharding, PartitionSpec
    sharded, in_names, out_names, out_avals, zero_outs, mesh = _get_runner()
    in_maps = make_in_maps(**np_inputs)
    concat_in = _concat_inputs(in_maps, in_names)
    sh = NamedSharding(mesh, PartitionSpec("core"))
    dev_in = [jax.device_put(a, sh) for a in concat_in]
    for a in dev_in:
        a.block_until_ready()
    best = float("inf")
    for _ in range(iters):
        concat_zeros = [jax.device_put(
            np.zeros((N_CORES * z.shape[0], *z.shape[1:]), z.dtype), sh)
            for z in zero_outs]
        for a in concat_zeros:
            a.block_until_ready()
        t0 = _time.perf_counter()
        out = sharded(*dev_in, *concat_zeros)
        for o in out:
            o.block_until_ready()
        best = min(best, _time.perf_counter() - t0)
    return best * 1e9


# revision 25
# speedup vs baseline: 1.0248x; 1.0248x over previous
"""BDH layer (sparse-attention GLA block) on 8 Trainium2 NeuronCores.

Sharding: data-parallel over B (2) x tensor-parallel over heads (4).
Core c handles batch c//4, head c%4. Each core computes its head's partial
decoder output yMLP; a 4-core AllReduce per batch group sums them and every
core finishes the final norms. Host gathers y from core 0 (b=0) / core 4 (b=1).

Self-contained: hardcodes the problem shapes (B=2, T=1024, D=256, NH=4,
N=4096), builds/compiles the Bass program once per process, and runs it via
run_bass_kernel_spmd on cores 0-7.
"""

import math
import numpy as np

import concourse.bass as bass
import concourse.mybir as mybir
import concourse.tile as tile
from concourse import bacc
from concourse.bass_utils import run_bass_kernel_spmd

F32 = mybir.dt.float32
F32R = mybir.dt.float32r
BF16 = mybir.dt.bfloat16
AF = mybir.ActivationFunctionType
ALU = mybir.AluOpType

# ---- problem constants ----
B, T, D, NH, N = 2, 1024, 256, 4, 4096
NK = N                      # per-head key width (one head per core)
C = 256                     # GLA chunk length used by this kernel (exact math)
ROPE_BASE = float(2 ** 18)
SCALE_BASE = 512.0
GATE_DIV = 1024.0
EPS = 1e-5
KT = NK // 128              # 32 k-tiles
NS = T // C                 # 4 sweeps
CC = C // 128               # 2
DT = D // 128               # 2
TT = T // 128               # 8
SCALE = N ** -0.5
N_CORES = 8
GROUPS = [[0, 1, 2, 3], [4, 5, 6, 7]]


def _rope_tables():
    d = 256
    inv_freq = 1.0 / (ROPE_BASE ** (np.arange(0, d, 2, dtype=np.float64) / d))
    t = np.arange(T, dtype=np.float64)
    freqs = t[:, None] * inv_freq[None, :]
    scale = (np.arange(0, d, 2, dtype=np.float64) + 0.4 * d) / (1.4 * d)
    power = (t - float(T // 2)) / SCALE_BASE
    sc = scale[None, :] ** power[:, None]
    cos = (np.cos(freqs) * sc).astype(np.float32)
    sin = (np.sin(freqs) * sc).astype(np.float32)
    return np.ascontiguousarray(cos.T), np.ascontiguousarray(sin.T)


def _build():
    nc = bacc.Bacc("TRN2", target_bir_lowering=False, debug=False,
                   num_devices=N_CORES)

    KS = KT // 4            # 4-ktile DMA slabs
    wenc = nc.dram_tensor("wenc", [KS, 128, 4 * DT * 128], F32R,
                          kind="ExternalInput")
    wgate = nc.dram_tensor("wgate", [KS, 128, 4 * DT * 128], F32R,
                           kind="ExternalInput")
    wencv = nc.dram_tensor("wencv", [KS, 128, 4 * DT * 128], F32R,
                           kind="ExternalInput")
    wdec = nc.dram_tensor("wdec", [KS, 128, 4 * D], F32R, kind="ExternalInput")
    xt = nc.dram_tensor("xt", [DT, 128, T], F32R, kind="ExternalInput")
    xv = nc.dram_tensor("xv", [TT, 128, D], F32R, kind="ExternalInput")
    cos_t = nc.dram_tensor("cos_t", [128, T], F32, kind="ExternalInput")
    sin_t = nc.dram_tensor("sin_t", [128, T], F32, kind="ExternalInput")
    triu = nc.dram_tensor("triu", [CC, 128, C], F32R, kind="ExternalInput")
    ident = nc.dram_tensor("ident", [128, 128], F32R, kind="ExternalInput")
    y_out = nc.dram_tensor("y", [TT, 128, D], F32, kind="ExternalOutput")

    ar_in = nc.dram_tensor("ar_in", [TT, 128, D], F32)
    ar_out = nc.dram_tensor("ar_out", [TT, 128, D], F32)

    ln_s = math.log(SCALE)
    relu_gate_scale = 1.0 / math.sqrt(GATE_DIV)

    with tile.TileContext(nc) as tc:
        with (
            tc.tile_pool(name="persist", bufs=1) as p_per,
            tc.tile_pool(name="wstream", bufs=2) as p_w,
            tc.tile_pool(name="wdecs", bufs=2) as p_wd,
            tc.tile_pool(name="tran", bufs=2) as p_t,
            tc.tile_pool(name="qk", bufs=6) as p_qk,
            tc.tile_pool(name="xsp", bufs=KT // 2 + 1) as p_xsp,
            tc.tile_pool(name="hpool", bufs=KT) as p_h,
            tc.tile_pool(name="small", bufs=2) as p_s,
            tc.tile_pool(name="ps_eg", bufs=2, space="PSUM") as ps_eg,
            tc.tile_pool(name="ps_at", bufs=1, space="PSUM") as ps_at,
            tc.tile_pool(name="ps_o", bufs=1, space="PSUM") as ps_o,
            tc.tile_pool(name="ps_upd", bufs=1, space="PSUM") as ps_upd,
        ):
            xt_sb = p_per.tile([128, DT * T], F32R, tag="xt")
            # first sweep's x^T slices first so enc GEMM can start asap
            for d_ in range(DT):
                nc.sync.dma_start(xt_sb[:, d_ * T: d_ * T + C],
                                  xt[d_, :, 0:C])
            for d_ in range(DT):
                nc.sync.dma_start(xt_sb[:, d_ * T + C:(d_ + 1) * T],
                                  xt[d_, :, C:T])
            cos_sb = p_per.tile([128, T], F32, tag="cos")
            nc.sync.dma_start(cos_sb[:], cos_t[:])
            sin_sb = p_per.tile([128, T], F32, tag="sin")
            nc.sync.dma_start(sin_sb[:], sin_t[:])
            xv_sb = p_per.tile([128, TT * D], F32R, tag="xv")
            for t_ in range(TT):
                nc.sync.dma_start(xv_sb[:, t_ * D:(t_ + 1) * D], xv[t_])
            triu_sb = p_per.tile([128, CC * C], F32R, tag="triu")
            for j in range(CC):
                nc.sync.dma_start(triu_sb[:, j * C:(j + 1) * C], triu[j])
            id_sb = p_per.tile([128, 128], F32R, tag="ident")
            nc.sync.dma_start(id_sb[:], ident[:])
            id_f32 = id_sb[:].bitcast(F32)

            yMLP = p_per.tile([128, TT * D], F32, tag="ymlp")
            lns_c = p_per.tile([128, 1], F32, tag="lns")
            nc.gpsimd.memset(lns_c[:], ln_s)
            eps_c = p_per.tile([128, 1], F32, tag="epsc")
            nc.gpsimd.memset(eps_c[:], EPS)

            h_tiles = [p_h.tile([128, D], F32R, tag="h", name=f"h{i}")
                       for i in range(KT)]

            def xtile(d_, s):
                return xt_sb[:, d_ * T + s * C: d_ * T + (s + 1) * C]

            def vtile(s, j):
                t_ = s * CC + j
                return xv_sb[:, t_ * D:(t_ + 1) * D]

            def final_tile(t_):
                ym = yMLP[:, t_ * D:(t_ + 1) * D]
                s1 = p_s.tile([128, 1], F32, tag="s1", name=f"fs1_{t_}")
                nc.vector.tensor_reduce(s1[:], ym, mybir.AxisListType.X, ALU.add)
                sq = p_t.tile([128, D], F32, tag="sqscr", name=f"fsq{t_}")
                s2 = p_s.tile([128, 1], F32, tag="s2", name=f"fs2_{t_}")
                nc.scalar.activation(sq[:], ym, AF.Square, accum_out=s2[:])
                mean = p_s.tile([128, 1], F32, tag="mean", name=f"fmn{t_}")
                nc.vector.tensor_scalar_mul(mean[:], s1[:], 1.0 / D)
                var = p_s.tile([128, 1], F32, tag="var", name=f"fvr{t_}")
                nc.vector.tensor_scalar_mul(var[:], s2[:], 1.0 / D)
                msq = p_s.tile([128, 1], F32, tag="msq", name=f"fmq{t_}")
                nc.vector.tensor_tensor(msq[:], mean[:], mean[:], ALU.mult)
                nc.vector.tensor_tensor(var[:], var[:], msq[:], ALU.subtract)
                std = p_s.tile([128, 1], F32, tag="std", name=f"fsd{t_}")
                nc.scalar.activation(std[:], var[:], AF.Sqrt, bias=eps_c[:])
                rstd = p_s.tile([128, 1], F32, tag="rstd", name=f"frs{t_}")
                nc.vector.reciprocal(rstd[:], std[:])
                nbias = p_s.tile([128, 1], F32, tag="nbias", name=f"fnb{t_}")
                nc.vector.tensor_tensor(nbias[:], mean[:], rstd[:], ALU.mult)
                nc.vector.tensor_scalar_mul(nbias[:], nbias[:], -1.0)
                ln = p_t.tile([128, D], F32, tag="ln", name=f"fln{t_}")
                nc.scalar.activation(ln[:], ym, AF.Identity,
                                     scale=rstd[:], bias=nbias[:])
                z = p_t.tile([128, D], F32, tag="z", name=f"fz{t_}")
                nc.vector.tensor_tensor(
                    z[:], ln[:], xv_sb[:, t_ * D:(t_ + 1) * D].bitcast(F32),
                    ALU.add)
                sq2 = p_t.tile([128, D], F32, tag="sqscr2", name=f"fq2{t_}")
                ms = p_s.tile([128, 1], F32, tag="ms", name=f"fms{t_}")
                nc.scalar.activation(sq2[:], z[:], AF.Square, accum_out=ms[:])
                nc.vector.tensor_scalar_mul(ms[:], ms[:], 1.0 / D)
                rms = p_s.tile([128, 1], F32, tag="rms", name=f"frm{t_}")
                nc.scalar.activation(rms[:], ms[:], AF.Sqrt, bias=eps_c[:])
                rr = p_s.tile([128, 1], F32, tag="rr", name=f"frr{t_}")
                nc.vector.reciprocal(rr[:], rms[:])
                yo = p_t.tile([128, D], F32, tag="yo", name=f"fy{t_}")
                nc.scalar.activation(yo[:], z[:], AF.Copy, scale=rr[:])
                nc.sync.dma_start(y_out[t_], yo[:])

            for s in range(NS):
                csl = slice(s * C, (s + 1) * C)
                at_ps = [ps_at.tile([128, C], F32, tag=f"at{j}", name=f"at{j}_{s}")
                         for j in range(CC)]
                o_ps = [ps_o.tile([128, D], F32, tag=f"o{g}", name=f"o{g}_{s}")
                        for g in range(CC)]

                xsp_tiles = [None] * KT
                qg_tiles = [None] * KT
                kg_tiles = [None] * KT

                for grp in range(KT // 4):
                    kts = [4 * grp + u for u in range(4)]
                    wesl = p_w.tile([128, 4 * DT * 128], F32R, tag="wenc",
                                    name=f"wes{s}_{grp}")
                    nc.sync.dma_start(wesl[:], wenc[grp])
                    wgsl = p_w.tile([128, 4 * DT * 128], F32R, tag="wgate",
                                    name=f"wgs{s}_{grp}")
                    nc.sync.dma_start(wgsl[:], wgate[grp])
                    bneg = p_t.tile([128, 4 * C], F32, tag="bneg",
                                    name=f"bn{s}_{grp}", bufs=2)
                    xsp_pair = [None, None]
                    for half in range(2):   # kt pairs within the group
                        pe2t = ps_eg.tile([128, 2 * C], F32, tag="eg",
                                          name=f"pe{s}_{grp}_{half}")
                        pg2t = ps_eg.tile([128, 2 * C], F32, tag="eg",
                                          name=f"pg{s}_{grp}_{half}")
                        pe2 = pe2t[:]
                        pg2 = pg2t[:]
                        for u in range(2):
                            ki = 2 * half + u
                            for d_ in range(DT):
                                w_sl = wesl[:, ki * 256 + d_ * 128:
                                            ki * 256 + d_ * 128 + 128]
                                nc.tensor.matmul(pe2[:, u * C:(u + 1) * C],
                                                 w_sl, xtile(d_, s),
                                                 start=(d_ == 0),
                                                 stop=(d_ == DT - 1))
                            for d_ in range(DT):
                                w_sl = wgsl[:, ki * 256 + d_ * 128:
                                            ki * 256 + d_ * 128 + 128]
                                nc.tensor.matmul(pg2[:, u * C:(u + 1) * C],
                                                 w_sl, xtile(d_, s),
                                                 start=(d_ == 0),
                                                 stop=(d_ == DT - 1))
                        raw = p_t.tile([128, 2 * C], F32, tag="raw",
                                       name=f"raw{s}_{grp}_{half}", bufs=2)
                        nc.scalar.activation(raw[:], pe2, AF.Copy)
                        xsp = p_xsp.tile([128, 2 * C], F32, tag="xsp",
                                         name=f"xsp{s}_{grp}_{half}")
                        nc.vector.scalar_tensor_tensor(
                            xsp[:], pe2, 0.0, raw[:], ALU.max, ALU.mult)
                        xsp_pair[half] = xsp
                        kt0 = 4 * grp + 2 * half
                        xsp_tiles[kt0] = xsp[:, 0:C]
                        xsp_tiles[kt0 + 1] = xsp[:, C:2 * C]
                        rg = p_t.tile([128, 2 * C], F32, tag="rg",
                                      name=f"rg{s}_{grp}_{half}", bufs=1)
                        nc.scalar.activation(rg[:], pg2, AF.Relu,
                                             scale=relu_gate_scale)
                        g2 = p_t.tile([128, 2 * C], F32, tag="g2",
                                      name=f"g2{s}_{grp}_{half}", bufs=1)
                        nc.scalar.activation(g2[:], rg[:], AF.Square)
                        for u in range(2):
                            nc.vector.tensor_tensor_scan(
                                bneg[:, (2 * half + u) * C:(2 * half + u + 1) * C],
                                g2[:, u * C:(u + 1) * C], g2[:, u * C:(u + 1) * C],
                                0.0, ALU.add, ALU.bypass)
                    eb = p_t.tile([128, 4 * C], F32R, tag="eb",
                                  name=f"eb{s}_{grp}", bufs=2)
                    nc.scalar.activation(eb[:], bneg[:], AF.Exp,
                                         scale=-1.0, bias=lns_c[:])
                    enb = p_t.tile([128, 4 * C], F32R, tag="enb",
                                   name=f"enb{s}_{grp}", bufs=2)
                    nc.scalar.activation(enb[:], bneg[:], AF.Exp)

                    cos_s, sin_s = cos_sb[:, csl], sin_sb[:, csl]
                    cos_b = cos_s.unsqueeze(1).broadcast_to([128, 2, C])
                    sin_b = sin_s.unsqueeze(1).broadcast_to([128, 2, C])
                    for half in range(2):
                        kt0 = 4 * grp + 2 * half
                        xsp = xsp_pair[half]
                        xsp3 = xsp[:].rearrange("p (a c) -> p a c", a=2)
                        mc = p_t.tile([128, 2 * C], F32, tag="mc",
                                      name=f"mc{s}_{grp}_{half}", bufs=1)
                        nc.vector.tensor_tensor(
                            mc[:].rearrange("p (a c) -> p a c", a=2),
                            xsp3, cos_b, ALU.mult)
                        ms_ = p_t.tile([128, 2 * C], F32, tag="ms_",
                                       name=f"msn{s}_{grp}_{half}", bufs=1)
                        nc.gpsimd.tensor_tensor(
                            ms_[:].rearrange("p (a c) -> p a c", a=2),
                            xsp3, sin_b, ALU.mult)
                        # mc = [x0*cos | x1*cos], ms_ = [x0*sin | x1*sin]
                        rot = p_qk.tile([128, 2 * C], F32, tag="rot",
                                        name=f"rot{s}_{grp}_{half}", bufs=2)
                        nc.gpsimd.tensor_tensor(rot[:, 0:C], mc[:, 0:C],
                                                ms_[:, C:2 * C], ALU.subtract)
                        nc.vector.tensor_tensor(rot[:, C:2 * C], ms_[:, 0:C],
                                                mc[:, C:2 * C], ALU.add)
                        ebsl = eb[:, 2 * half * C:(2 * half + 2) * C]
                        enbsl = enb[:, 2 * half * C:(2 * half + 2) * C]
                        qg2 = p_qk.tile([128, 2 * C], F32R, tag="qg",
                                        name=f"qg{s}_{grp}_{half}", bufs=2)
                        nc.vector.tensor_tensor(qg2[:], rot[:], ebsl, ALU.mult)
                        kg2 = p_qk.tile([128, 2 * C], F32R, tag="kg",
                                        name=f"kg{s}_{grp}_{half}", bufs=2)
                        nc.gpsimd.tensor_tensor(kg2[:], rot[:], enbsl, ALU.mult)
                        qg_tiles[kt0] = qg2[:, 0:C]
                        qg_tiles[kt0 + 1] = qg2[:, C:2 * C]
                        kg_tiles[kt0] = kg2[:, 0:C]
                        kg_tiles[kt0 + 1] = kg2[:, C:2 * C]

                        # ---- GLA for this pair ----
                        if s < NS - 1:
                            tp = ps_upd.tile([128, 2 * C], F32, tag="tp",
                                             name=f"tp{s}_{grp}_{half}")
                        for u in range(2):
                            kt = kt0 + u
                            qg, kg = qg_tiles[kt], kg_tiles[kt]
                            first, last = (kt == 0), (kt == KT - 1)
                            for j in range(CC):
                                nc.tensor.matmul(at_ps[j][:],
                                                 kg[:, j * 128:(j + 1) * 128],
                                                 qg, start=first, stop=last)
                            if s > 0:
                                for g in range(CC):
                                    nc.tensor.matmul(
                                        o_ps[g][:], qg[:, g * 128:(g + 1) * 128],
                                        h_tiles[kt][:], start=first, stop=False)
                            if s < NS - 1:
                                for j in range(CC):
                                    nc.tensor.transpose(
                                        tp[:, (2 * u + j) * 128:
                                           (2 * u + j + 1) * 128],
                                        kg[:, j * 128:(j + 1) * 128].bitcast(F32),
                                        id_f32)
                        if s < NS - 1:
                            kgdbT = p_t.tile([128, 2 * C], F32R, tag="kgdbT",
                                             name=f"kgT{s}_{grp}_{half}", bufs=1)
                            nc.scalar.activation(kgdbT[:], tp[:], AF.Copy)
                            dbv = p_s.tile([128, 2], F32, tag="dbv",
                                           name=f"dbv{s}_{grp}_{half}", bufs=4)
                            nc.vector.tensor_scalar_mul(
                                dbv[:], ebsl.bitcast(F32)[:, C - 1:2 * C:C],
                                1.0 / SCALE)
                            for u in range(2):
                                kt = kt0 + u
                                upd = ps_upd.tile([128, D], F32, tag="upd",
                                                  name=f"up{s}_{kt}")
                                for j in range(CC):
                                    nc.tensor.matmul(
                                        upd[:],
                                        kgdbT[:, (2 * u + j) * 128:
                                              (2 * u + j + 1) * 128],
                                        vtile(s, j), start=(j == 0),
                                        stop=(s == 0 and j == CC - 1))
                                if s > 0:
                                    nc.tensor.matmul(upd[:], id_sb[:],
                                                     h_tiles[kt][:],
                                                     start=False, stop=True)
                                nc.scalar.activation(h_tiles[kt][:], upd[:],
                                                     AF.Copy,
                                                     scale=dbv[:, u:u + 1])

                at_sb = []
                for j in range(CC):
                    m = p_t.tile([128, C], F32R, tag=f"atsb{j}",
                                 name=f"atsb{j}_{s}")
                    nc.vector.tensor_tensor(
                        m[:], at_ps[j][:],
                        triu_sb[:, j * C:(j + 1) * C].bitcast(F32), ALU.mult)
                    at_sb.append(m)
                for g in range(CC):
                    for j in range(CC):
                        nc.tensor.matmul(o_ps[g][:],
                                         at_sb[j][:, g * 128:(g + 1) * 128],
                                         vtile(s, j),
                                         start=(s == 0 and j == 0),
                                         stop=(j == CC - 1))

                ykvt = p_s.tile([128, CC * C], F32R, tag="ykvt", name=f"ykvt{s}", bufs=1)
                for g in range(CC):
                    o_t = o_ps[g]
                    s1 = p_s.tile([128, 1], F32, tag="s1", name=f"s1_{s}_{g}")
                    nc.vector.tensor_reduce(s1[:], o_t[:], mybir.AxisListType.X,
                                            ALU.add)
                    sq = p_t.tile([128, D], F32, tag="sqscr", name=f"sq{s}_{g}")
                    s2 = p_s.tile([128, 1], F32, tag="s2", name=f"s2_{s}_{g}")
                    nc.scalar.activation(sq[:], o_t[:], AF.Square, accum_out=s2[:])
                    mean = p_s.tile([128, 1], F32, tag="mean", name=f"mn{s}_{g}")
                    nc.vector.tensor_scalar_mul(mean[:], s1[:], 1.0 / D)
                    var = p_s.tile([128, 1], F32, tag="var", name=f"vr{s}_{g}")
                    nc.vector.tensor_scalar_mul(var[:], s2[:], 1.0 / D)
                    msq = p_s.tile([128, 1], F32, tag="msq", name=f"mq{s}_{g}")
                    nc.vector.tensor_tensor(msq[:], mean[:], mean[:], ALU.mult)
                    nc.vector.tensor_tensor(var[:], var[:], msq[:], ALU.subtract)
                    std = p_s.tile([128, 1], F32, tag="std", name=f"sd{s}_{g}")
                    nc.scalar.activation(std[:], var[:], AF.Sqrt, bias=eps_c[:])
                    rstd = p_s.tile([128, 1], F32, tag="rstd", name=f"rs{s}_{g}")
                    nc.vector.reciprocal(rstd[:], std[:])
                    nbias = p_s.tile([128, 1], F32, tag="nbias", name=f"nb{s}_{g}")
                    nc.vector.tensor_tensor(nbias[:], mean[:], rstd[:], ALU.mult)
                    nc.vector.tensor_scalar_mul(nbias[:], nbias[:], -1.0)
                    ykv = p_t.tile([128, D], F32, tag="ykv", name=f"ykv{s}_{g}")
                    nc.scalar.activation(ykv[:], o_t[:], AF.Identity,
                                         scale=rstd[:], bias=nbias[:])
                    tp2 = ps_upd.tile([128, C], F32, tag="tp", name=f"tpy{s}_{g}")
                    for d_ in range(DT):
                        nc.tensor.transpose(tp2[:, d_ * 128:(d_ + 1) * 128],
                                            ykv[:, d_ * 128:(d_ + 1) * 128], id_f32)
                    for d_ in range(DT):
                        nc.scalar.activation(
                            ykvt[:, d_ * C + g * 128: d_ * C + (g + 1) * 128],
                            tp2[:, d_ * 128:(d_ + 1) * 128], AF.Copy)

                dec_ps = [ps_at.tile([128, D], F32, tag=f"at{g}",
                                     name=f"dc{g}_{s}") for g in range(CC)]
                for kp in range(KT // 2):     # kt pairs
                    kt0 = 2 * kp
                    if kt0 % 4 == 0:
                        ks = kt0 // 4
                        wvsl = p_w.tile([128, 4 * DT * 128], F32R, tag="wencv",
                                        name=f"wvs{s}_{ks}")
                        nc.sync.dma_start(wvsl[:], wencv[ks])
                        wdsl = p_wd.tile([128, 4 * D], F32R, tag="wdec",
                                         name=f"wds{s}_{ks}")
                        nc.sync.dma_start(wdsl[:], wdec[ks])
                    evps = ps_upd.tile([128, 2 * C], F32,
                                       tag=("tp" if kp % 2 == 0 else "upd"),
                                       name=f"pv{s}_{kp}")
                    for u in range(2):
                        ki = (kt0 + u) % 4
                        for d_ in range(DT):
                            nc.tensor.matmul(
                                evps[:, u * C:(u + 1) * C],
                                wvsl[:, ki * 256 + d_ * 128:
                                     ki * 256 + d_ * 128 + 128],
                                ykvt[:, d_ * C:(d_ + 1) * C],
                                start=(d_ == 0), stop=(d_ == DT - 1))
                    t1 = p_t.tile([128, 2 * C], F32, tag="t1",
                                  name=f"t1_{s}_{kp}", bufs=1)
                    xsp_pair_ap = xsp_tiles[kt0].tensor.ap()[:, 0:2 * C]
                    nc.vector.scalar_tensor_tensor(
                        t1[:], evps[:], 0.0, xsp_pair_ap, ALU.max, ALU.mult)
                    xy = p_t.tile([128, 2 * C], F32R, tag="xy",
                                  name=f"xy{s}_{kp}")
                    nc.vector.scalar_tensor_tensor(
                        xy[:], evps[:], 0.0, t1[:], ALU.bypass, ALU.mult)
                    for u in range(2):
                        kt = kt0 + u
                        ki = kt % 4
                        for g in range(CC):
                            nc.tensor.matmul(
                                dec_ps[g][:],
                                xy[:, u * C + g * 128: u * C + (g + 1) * 128],
                                wdsl[:, ki * D:(ki + 1) * D],
                                start=(kt == 0), stop=(kt == KT - 1))
                for g in range(CC):
                    t_ = s * CC + g
                    nc.scalar.activation(yMLP[:, t_ * D:(t_ + 1) * D],
                                         dec_ps[g][:], AF.Copy)
                t0 = s * CC
                if N_CORES > 1:
                    for g in range(CC):
                        t_ = t0 + g
                        nc.sync.dma_start(ar_in[t_],
                                          yMLP[:, t_ * D:(t_ + 1) * D])
                    nc.gpsimd.collective_compute(
                        "AllReduce", ALU.add, replica_groups=GROUPS,
                        ins=[ar_in[t0:t0 + CC]], outs=[ar_out[t0:t0 + CC]])
                    for g in range(CC):
                        t_ = t0 + g
                        nc.sync.dma_start(yMLP[:, t_ * D:(t_ + 1) * D],
                                          ar_out[t_])


            for t_ in range(TT):
                final_tile(t_)

    nc.compile()
    return nc


def _tile_w(W):
    # (D, NK) -> (KT//4, 128, 4*DT*128): slab ks holds k-tiles 4ks..4ks+3,
    # column order (ki, d, c) matching the SBUF slab layout.
    W = np.asarray(W, dtype=np.float32)
    a = W.reshape(DT, 128, KT // 4, 4, 128).transpose(2, 1, 3, 0, 4)
    return np.ascontiguousarray(a.reshape(KT // 4, 128, 4 * DT * 128))


def _tile_wdec(W):
    # (NK, D) -> (KT//4, 128, 4*D)
    W = np.asarray(W, dtype=np.float32)
    a = W.reshape(KT // 4, 4, 128, D).transpose(0, 2, 1, 3)
    return np.ascontiguousarray(a.reshape(KT // 4, 128, 4 * D))


_STATE = {}


def _get_nc():
    if "nc" not in _STATE:
        _STATE["nc"] = _build()
    return _STATE["nc"]


def _core_in_map(x_b, W_enc_h, W_gate_h, W_encv_h, W_dec_h, consts):
    cos_t, sin_t, triu, ident = consts
    xT = np.ascontiguousarray(x_b.T)
    return {
        "wenc": _tile_w(W_enc_h),
        "wgate": _tile_w(W_gate_h),
        "wencv": _tile_w(W_encv_h),
        "wdec": _tile_wdec(W_dec_h),
        "xt": np.ascontiguousarray(xT.reshape(DT, 128, T)),
        "xv": np.ascontiguousarray(x_b.reshape(TT, 128, D)),
        "cos_t": cos_t, "sin_t": sin_t, "triu": triu, "ident": ident,
    }


def make_in_maps(x, W_enc, W_gate, W_dec, W_encv):
    x = np.asarray(x, dtype=np.float32)
    W_enc = np.asarray(W_enc, dtype=np.float32)
    W_gate = np.asarray(W_gate, dtype=np.float32)
    W_dec = np.asarray(W_dec, dtype=np.float32)
    W_encv = np.asarray(W_encv, dtype=np.float32)

    cos_t, sin_t = _rope_tables()
    triu = np.zeros((CC, 128, C), dtype=np.float32)
    for j in range(CC):
        for p in range(128):
            triu[j, p, j * 128 + p:] = 1.0
    ident = np.eye(128, dtype=np.float32)
    consts = (cos_t, sin_t, triu, ident)

    in_maps = []
    for c in range(N_CORES):
        b, h = c // 4, c % 4
        nsl = slice(h * N, (h + 1) * N)
        in_maps.append(_core_in_map(
            x[b], W_enc[:, nsl], W_gate[:, nsl], W_encv[h], W_dec[nsl, :],
            consts))
    return in_maps


def _get_runner():
    """Cached jitted SPMD executable mirroring bass2jax.run_bass_via_pjrt, so
    repeated kernel() calls skip re-tracing/recompiling."""
    if "runner" in _STATE:
        return _STATE["runner"]
    import jax
    import concourse.mybir as mb
    from concourse import bass2jax as b2j
    from jax.experimental.shard_map import shard_map
    from jax.sharding import Mesh, PartitionSpec

    nc = _get_nc()
    b2j.install_neuronx_cc_hook()
    partition_name = (nc.partition_id_tensor.name
                      if nc.partition_id_tensor else None)
    in_names, out_names, out_avals, zero_outs = [], [], [], []
    for alloc in nc.m.functions[0].allocations:
        if not isinstance(alloc, mb.MemoryLocationSet):
            continue
        name = alloc.memorylocations[0].name
        if alloc.kind == "ExternalInput":
            if name != partition_name:
                in_names.append(name)
        elif alloc.kind == "ExternalOutput":
            shape = tuple(alloc.tensor_shape)
            dtype = mb.dt.np(alloc.dtype)
            out_names.append(name)
            out_avals.append(jax.core.ShapedArray(shape, dtype))
            zero_outs.append(np.zeros(shape, dtype))
    n_params = len(in_names)
    all_names = in_names + out_names
    if partition_name is not None:
        all_names = all_names + [partition_name]
    donate = tuple(range(n_params, n_params + len(out_names)))

    def _body(*args):
        operands = list(args)
        if partition_name is not None:
            operands.append(b2j.partition_id_tensor())
        return tuple(b2j._bass_exec_p.bind(
            *operands,
            out_avals=tuple(out_avals),
            in_names=tuple(all_names),
            out_names=tuple(out_names),
            lowering_input_output_aliases=(),
            sim_require_finite=True,
            sim_require_nnan=True,
            nc=nc,
        ))

    devices = jax.devices()[:N_CORES]
    mesh = Mesh(np.asarray(devices), ("core",))
    in_specs = (PartitionSpec("core"),) * (n_params + len(out_names))
    out_specs = (PartitionSpec("core"),) * len(out_names)
    sharded = jax.jit(
        shard_map(_body, mesh=mesh, in_specs=in_specs, out_specs=out_specs,
                  check_rep=False),
        donate_argnums=donate, keep_unused=True)
    _STATE["runner"] = (sharded, in_names, out_names, out_avals, zero_outs, mesh)
    return _STATE["runner"]


def _concat_inputs(in_maps, in_names):
    return [np.concatenate([np.asarray(in_maps[c][nm]) for c in range(N_CORES)],
                           axis=0) for nm in in_names]


def _run(in_maps):
    sharded, in_names, out_names, out_avals, zero_outs, mesh = _get_runner()
    concat_in = _concat_inputs(in_maps, in_names)
    concat_zeros = [np.zeros((N_CORES * z.shape[0], *z.shape[1:]), z.dtype)
                    for z in zero_outs]
    out_arrs = sharded(*concat_in, *concat_zeros)
    return {name: np.asarray(out_arrs[i]).reshape(N_CORES, *out_avals[i].shape)
            for i, name in enumerate(out_names)}


def kernel(x, W_enc, W_gate, W_dec, W_encv):
    in_maps = make_in_maps(x, W_enc, W_gate, W_dec, W_encv)
    outs = _run(in_maps)
    y0 = outs["y"][0].reshape(T, D)
    y1 = outs["y"][4].reshape(T, D)
    return np.stack([y0, y1]).astype(np.float32)


def time_device_exec(np_inputs, iters=10):
    """Best wall-clock (ns) of the device execution with inputs pre-staged on
    device; excludes host prep and output conversion."""
    import time as _time
    import jax
    from jax.sharding import NamedSharding, PartitionSpec
    sharded, in_names, out_names, out_avals, zero_outs, mesh = _get_runner()
    in_maps = make_in_maps(**np_inputs)
    concat_in = _concat_inputs(in_maps, in_names)
    sh = NamedSharding(mesh, PartitionSpec("core"))
    dev_in = [jax.device_put(a, sh) for a in concat_in]
    for a in dev_in:
        a.block_until_ready()
    best = float("inf")
    for _ in range(iters):
        concat_zeros = [jax.device_put(
            np.zeros((N_CORES * z.shape[0], *z.shape[1:]), z.dtype), sh)
            for z in zero_outs]
        for a in concat_zeros:
            a.block_until_ready()
        t0 = _time.perf_counter()
        out = sharded(*dev_in, *concat_zeros)
        for o in out:
            o.block_until_ready()
        best = min(best, _time.perf_counter() - t0)
    return best * 1e9


# revision 31
# speedup vs baseline: 1.2615x; 1.2309x over previous
"""BDH layer (sparse-attention GLA block) on 8 Trainium2 NeuronCores.

Sharding: data-parallel over B (2) x tensor-parallel over heads (4).
Core c handles batch c//4, head c%4. Each core computes its head's partial
decoder output yMLP; a 4-core AllReduce per batch group sums them and every
core finishes the final norms. Host gathers y from core 0 (b=0) / core 4 (b=1).

Self-contained: hardcodes the problem shapes (B=2, T=1024, D=256, NH=4,
N=4096), builds/compiles the Bass program once per process, and runs it via
run_bass_kernel_spmd on cores 0-7.
"""

import math
import numpy as np

import concourse.bass as bass
import concourse.mybir as mybir
import concourse.tile as tile
from concourse import bacc
from concourse.bass_utils import run_bass_kernel_spmd

F32 = mybir.dt.float32
F32R = mybir.dt.float32r
BF16 = mybir.dt.bfloat16
AF = mybir.ActivationFunctionType
ALU = mybir.AluOpType

# ---- problem constants ----
B, T, D, NH, N = 2, 1024, 256, 4, 4096
NK = N                      # per-head key width (one head per core)
C = 256                     # GLA chunk length used by this kernel (exact math)
ROPE_BASE = float(2 ** 18)
SCALE_BASE = 512.0
GATE_DIV = 1024.0
EPS = 1e-5
KT = NK // 128              # 32 k-tiles
NS = T // C                 # 4 sweeps
CC = C // 128               # 2
DT = D // 128               # 2
TT = T // 128               # 8
SCALE = N ** -0.5
N_CORES = 8
GROUPS = [[0, 1, 2, 3], [4, 5, 6, 7]]


def _rope_tables():
    d = 256
    inv_freq = 1.0 / (ROPE_BASE ** (np.arange(0, d, 2, dtype=np.float64) / d))
    t = np.arange(T, dtype=np.float64)
    freqs = t[:, None] * inv_freq[None, :]
    scale = (np.arange(0, d, 2, dtype=np.float64) + 0.4 * d) / (1.4 * d)
    power = (t - float(T // 2)) / SCALE_BASE
    sc = scale[None, :] ** power[:, None]
    cos = (np.cos(freqs) * sc).astype(np.float32)
    sin = (np.sin(freqs) * sc).astype(np.float32)
    return np.ascontiguousarray(cos.T), np.ascontiguousarray(sin.T)


def _build():
    nc = bacc.Bacc("TRN2", target_bir_lowering=False, debug=False,
                   num_devices=N_CORES)

    KS = KT // 4            # 4-ktile DMA slabs
    wenc = nc.dram_tensor("wenc", [KS, 128, 4 * DT * 128], F32R,
                          kind="ExternalInput")
    wgate = nc.dram_tensor("wgate", [KS, 128, 4 * DT * 128], F32R,
                           kind="ExternalInput")
    wencv = nc.dram_tensor("wencv", [KS, 128, 4 * DT * 128], F32R,
                           kind="ExternalInput")
    wdec = nc.dram_tensor("wdec", [KS, 128, 4 * D], F32R, kind="ExternalInput")
    xt = nc.dram_tensor("xt", [DT, 128, T], F32R, kind="ExternalInput")
    xv = nc.dram_tensor("xv", [TT, 128, D], F32R, kind="ExternalInput")
    cos_t = nc.dram_tensor("cos_t", [128, T], F32, kind="ExternalInput")
    sin_t = nc.dram_tensor("sin_t", [128, T], F32, kind="ExternalInput")
    triu = nc.dram_tensor("triu", [CC, 128, C], F32R, kind="ExternalInput")
    ident = nc.dram_tensor("ident", [128, 128], F32R, kind="ExternalInput")
    y_out = nc.dram_tensor("y", [TT, 128, D], F32, kind="ExternalOutput")

    ar_in = nc.dram_tensor("ar_in", [TT, 128, D], F32)
    ar_out = nc.dram_tensor("ar_out", [TT, 128, D], F32)

    ln_s = math.log(SCALE)
    relu_gate_scale = 1.0 / math.sqrt(GATE_DIV)

    with tile.TileContext(nc) as tc:
        with (
            tc.tile_pool(name="persist", bufs=1) as p_per,
            tc.tile_pool(name="wstream", bufs=2) as p_w,
            tc.tile_pool(name="wdecs", bufs=2) as p_wd,
            tc.tile_pool(name="tran", bufs=2) as p_t,
            tc.tile_pool(name="qk", bufs=6) as p_qk,
            tc.tile_pool(name="xsp", bufs=KT // 2 + 1) as p_xsp,
            tc.tile_pool(name="hpool", bufs=KT) as p_h,
            tc.tile_pool(name="small", bufs=2) as p_s,
            tc.tile_pool(name="ps_eg", bufs=2, space="PSUM") as ps_eg,
            tc.tile_pool(name="ps_at", bufs=1, space="PSUM") as ps_at,
            tc.tile_pool(name="ps_o", bufs=1, space="PSUM") as ps_o,
            tc.tile_pool(name="ps_upd", bufs=1, space="PSUM") as ps_upd,
        ):
            xt_sb = p_per.tile([128, DT * T], F32R, tag="xt")
            # first sweep's x^T slices first so enc GEMM can start asap
            for d_ in range(DT):
                nc.sync.dma_start(xt_sb[:, d_ * T: d_ * T + C],
                                  xt[d_, :, 0:C])
            for d_ in range(DT):
                nc.sync.dma_start(xt_sb[:, d_ * T + C:(d_ + 1) * T],
                                  xt[d_, :, C:T])
            cos_sb = p_per.tile([128, T], F32, tag="cos")
            nc.sync.dma_start(cos_sb[:], cos_t[:])
            sin_sb = p_per.tile([128, T], F32, tag="sin")
            nc.sync.dma_start(sin_sb[:], sin_t[:])
            xv_sb = p_per.tile([128, TT * D], F32R, tag="xv")
            for t_ in range(TT):
                nc.sync.dma_start(xv_sb[:, t_ * D:(t_ + 1) * D], xv[t_])
            triu_sb = p_per.tile([128, CC * C], F32R, tag="triu")
            for j in range(CC):
                nc.sync.dma_start(triu_sb[:, j * C:(j + 1) * C], triu[j])
            id_sb = p_per.tile([128, 128], F32R, tag="ident")
            nc.sync.dma_start(id_sb[:], ident[:])
            id_f32 = id_sb[:].bitcast(F32)

            yMLP = p_per.tile([128, TT * D], F32, tag="ymlp")
            lns_c = p_per.tile([128, 1], F32, tag="lns")
            nc.gpsimd.memset(lns_c[:], ln_s)
            eps_c = p_per.tile([128, 1], F32, tag="epsc")
            nc.gpsimd.memset(eps_c[:], EPS)

            h_tiles = [p_h.tile([128, D], F32R, tag="h", name=f"h{i}")
                       for i in range(KT)]

            def xtile(d_, s):
                return xt_sb[:, d_ * T + s * C: d_ * T + (s + 1) * C]

            def vtile(s, j):
                t_ = s * CC + j
                return xv_sb[:, t_ * D:(t_ + 1) * D]

            def final_tile(t_):
                ym = yMLP[:, t_ * D:(t_ + 1) * D]
                s1 = p_s.tile([128, 1], F32, tag="s1", name=f"fs1_{t_}")
                nc.vector.tensor_reduce(s1[:], ym, mybir.AxisListType.X, ALU.add)
                sq = p_t.tile([128, D], F32, tag="sqscr", name=f"fsq{t_}")
                s2 = p_s.tile([128, 1], F32, tag="s2", name=f"fs2_{t_}")
                nc.scalar.activation(sq[:], ym, AF.Square, accum_out=s2[:])
                mean = p_s.tile([128, 1], F32, tag="mean", name=f"fmn{t_}")
                nc.vector.tensor_scalar_mul(mean[:], s1[:], 1.0 / D)
                var = p_s.tile([128, 1], F32, tag="var", name=f"fvr{t_}")
                nc.vector.tensor_scalar_mul(var[:], s2[:], 1.0 / D)
                msq = p_s.tile([128, 1], F32, tag="msq", name=f"fmq{t_}")
                nc.vector.tensor_tensor(msq[:], mean[:], mean[:], ALU.mult)
                nc.vector.tensor_tensor(var[:], var[:], msq[:], ALU.subtract)
                std = p_s.tile([128, 1], F32, tag="std", name=f"fsd{t_}")
                nc.scalar.activation(std[:], var[:], AF.Sqrt, bias=eps_c[:])
                rstd = p_s.tile([128, 1], F32, tag="rstd", name=f"frs{t_}")
                nc.vector.reciprocal(rstd[:], std[:])
                nbias = p_s.tile([128, 1], F32, tag="nbias", name=f"fnb{t_}")
                nc.vector.tensor_tensor(nbias[:], mean[:], rstd[:], ALU.mult)
                nc.vector.tensor_scalar_mul(nbias[:], nbias[:], -1.0)
                ln = p_t.tile([128, D], F32, tag="ln", name=f"fln{t_}")
                nc.scalar.activation(ln[:], ym, AF.Identity,
                                     scale=rstd[:], bias=nbias[:])
                z = p_t.tile([128, D], F32, tag="z", name=f"fz{t_}")
                nc.vector.tensor_tensor(
                    z[:], ln[:], xv_sb[:, t_ * D:(t_ + 1) * D].bitcast(F32),
                    ALU.add)
                sq2 = p_t.tile([128, D], F32, tag="sqscr2", name=f"fq2{t_}")
                ms = p_s.tile([128, 1], F32, tag="ms", name=f"fms{t_}")
                nc.scalar.activation(sq2[:], z[:], AF.Square, accum_out=ms[:])
                nc.vector.tensor_scalar_mul(ms[:], ms[:], 1.0 / D)
                rms = p_s.tile([128, 1], F32, tag="rms", name=f"frm{t_}")
                nc.scalar.activation(rms[:], ms[:], AF.Sqrt, bias=eps_c[:])
                rr = p_s.tile([128, 1], F32, tag="rr", name=f"frr{t_}")
                nc.vector.reciprocal(rr[:], rms[:])
                yo = p_t.tile([128, D], F32, tag="yo", name=f"fy{t_}")
                nc.scalar.activation(yo[:], z[:], AF.Copy, scale=rr[:])
                nc.sync.dma_start(y_out[t_], yo[:])

            for s in range(NS):
                csl = slice(s * C, (s + 1) * C)
                at_ps = [ps_at.tile([128, C], F32, tag=f"at{j}", name=f"at{j}_{s}")
                         for j in range(CC)]
                o_ps = [ps_o.tile([128, D], F32, tag=f"o{g}", name=f"o{g}_{s}")
                        for g in range(CC)]

                xsp_tiles = [None] * KT
                qg_tiles = [None] * KT
                kg_tiles = [None] * KT

                for grp in range(KT // 4):
                    kts = [4 * grp + u for u in range(4)]
                    wesl = p_w.tile([128, 4 * DT * 128], F32R, tag="wenc",
                                    name=f"wes{s}_{grp}")
                    nc.sync.dma_start(wesl[:], wenc[grp])
                    wgsl = p_w.tile([128, 4 * DT * 128], F32R, tag="wgate",
                                    name=f"wgs{s}_{grp}")
                    nc.sync.dma_start(wgsl[:], wgate[grp])
                    bneg = p_t.tile([128, 4 * C], F32, tag="bneg",
                                    name=f"bn{s}_{grp}", bufs=2)
                    xsp_pair = [None, None]
                    for half in range(2):   # kt pairs within the group
                        pe2t = ps_eg.tile([128, 2 * C], F32, tag="eg",
                                          name=f"pe{s}_{grp}_{half}")
                        pg2t = ps_eg.tile([128, 2 * C], F32, tag="eg",
                                          name=f"pg{s}_{grp}_{half}")
                        pe2 = pe2t[:]
                        pg2 = pg2t[:]
                        for u in range(2):
                            ki = 2 * half + u
                            for d_ in range(DT):
                                w_sl = wesl[:, ki * 256 + d_ * 128:
                                            ki * 256 + d_ * 128 + 128]
                                nc.tensor.matmul(pe2[:, u * C:(u + 1) * C],
                                                 w_sl, xtile(d_, s),
                                                 start=(d_ == 0),
                                                 stop=(d_ == DT - 1))
                            for d_ in range(DT):
                                w_sl = wgsl[:, ki * 256 + d_ * 128:
                                            ki * 256 + d_ * 128 + 128]
                                nc.tensor.matmul(pg2[:, u * C:(u + 1) * C],
                                                 w_sl, xtile(d_, s),
                                                 start=(d_ == 0),
                                                 stop=(d_ == DT - 1))
                        raw = p_t.tile([128, 2 * C], F32, tag="raw",
                                       name=f"raw{s}_{grp}_{half}", bufs=2)
                        nc.scalar.activation(raw[:], pe2, AF.Copy)
                        xsp = p_xsp.tile([128, 2 * C], F32, tag="xsp",
                                         name=f"xsp{s}_{grp}_{half}")
                        nc.vector.scalar_tensor_tensor(
                            xsp[:], pe2, 0.0, raw[:], ALU.max, ALU.mult)
                        xsp_pair[half] = xsp
                        kt0 = 4 * grp + 2 * half
                        xsp_tiles[kt0] = xsp[:, 0:C]
                        xsp_tiles[kt0 + 1] = xsp[:, C:2 * C]
                        rg = p_t.tile([128, 2 * C], F32, tag="rg",
                                      name=f"rg{s}_{grp}_{half}", bufs=1)
                        nc.scalar.activation(rg[:], pg2, AF.Relu,
                                             scale=relu_gate_scale)
                        g2 = p_t.tile([128, 2 * C], F32, tag="g2",
                                      name=f"g2{s}_{grp}_{half}", bufs=1)
                        nc.scalar.activation(g2[:], rg[:], AF.Square)
                        for u in range(2):
                            nc.vector.tensor_tensor_scan(
                                bneg[:, (2 * half + u) * C:(2 * half + u + 1) * C],
                                g2[:, u * C:(u + 1) * C], g2[:, u * C:(u + 1) * C],
                                0.0, ALU.add, ALU.bypass)
                    eb = p_t.tile([128, 4 * C], F32R, tag="eb",
                                  name=f"eb{s}_{grp}", bufs=2)
                    nc.scalar.activation(eb[:], bneg[:], AF.Exp,
                                         scale=-1.0, bias=lns_c[:])
                    enb = p_t.tile([128, 4 * C], F32R, tag="enb",
                                   name=f"enb{s}_{grp}", bufs=2)
                    nc.scalar.activation(enb[:], bneg[:], AF.Exp)

                    cos_s, sin_s = cos_sb[:, csl], sin_sb[:, csl]
                    cos_b = cos_s.unsqueeze(1).broadcast_to([128, 2, C])
                    sin_b = sin_s.unsqueeze(1).broadcast_to([128, 2, C])
                    for half in range(2):
                        kt0 = 4 * grp + 2 * half
                        xsp = xsp_pair[half]
                        xsp3 = xsp[:].rearrange("p (a c) -> p a c", a=2)
                        mc = p_t.tile([128, 2 * C], F32, tag="mc",
                                      name=f"mc{s}_{grp}_{half}", bufs=1)
                        nc.vector.tensor_tensor(
                            mc[:].rearrange("p (a c) -> p a c", a=2),
                            xsp3, cos_b, ALU.mult)
                        ms_ = p_t.tile([128, 2 * C], F32, tag="ms_",
                                       name=f"msn{s}_{grp}_{half}", bufs=1)
                        nc.gpsimd.tensor_tensor(
                            ms_[:].rearrange("p (a c) -> p a c", a=2),
                            xsp3, sin_b, ALU.mult)
                        # mc = [x0*cos | x1*cos], ms_ = [x0*sin | x1*sin]
                        rot = p_qk.tile([128, 2 * C], F32, tag="rot",
                                        name=f"rot{s}_{grp}_{half}", bufs=2)
                        nc.gpsimd.tensor_tensor(rot[:, 0:C], mc[:, 0:C],
                                                ms_[:, C:2 * C], ALU.subtract)
                        nc.vector.tensor_tensor(rot[:, C:2 * C], ms_[:, 0:C],
                                                mc[:, C:2 * C], ALU.add)
                        ebsl = eb[:, 2 * half * C:(2 * half + 2) * C]
                        enbsl = enb[:, 2 * half * C:(2 * half + 2) * C]
                        qg2 = p_qk.tile([128, 2 * C], F32R, tag="qg",
                                        name=f"qg{s}_{grp}_{half}", bufs=2)
                        nc.vector.tensor_tensor(qg2[:], rot[:], ebsl, ALU.mult)
                        kg2 = p_qk.tile([128, 2 * C], F32R, tag="kg",
                                        name=f"kg{s}_{grp}_{half}", bufs=2)
                        nc.gpsimd.tensor_tensor(kg2[:], rot[:], enbsl, ALU.mult)
                        qg_tiles[kt0] = qg2[:, 0:C]
                        qg_tiles[kt0 + 1] = qg2[:, C:2 * C]
                        kg_tiles[kt0] = kg2[:, 0:C]
                        kg_tiles[kt0 + 1] = kg2[:, C:2 * C]

                        # ---- GLA for this pair ----
                        if s < NS - 1:
                            tp = ps_upd.tile([128, 2 * C], F32, tag="tp",
                                             name=f"tp{s}_{grp}_{half}")
                        for u in range(2):
                            kt = kt0 + u
                            qg, kg = qg_tiles[kt], kg_tiles[kt]
                            first, last = (kt == 0), (kt == KT - 1)
                            for j in range(CC):
                                nc.tensor.matmul(at_ps[j][:],
                                                 kg[:, j * 128:(j + 1) * 128],
                                                 qg, start=first, stop=last)
                            if s > 0:
                                for g in range(CC):
                                    nc.tensor.matmul(
                                        o_ps[g][:], qg[:, g * 128:(g + 1) * 128],
                                        h_tiles[kt][:], start=first, stop=False)
                            if s < NS - 1:
                                for j in range(CC):
                                    nc.tensor.transpose(
                                        tp[:, (2 * u + j) * 128:
                                           (2 * u + j + 1) * 128],
                                        kg[:, j * 128:(j + 1) * 128].bitcast(F32),
                                        id_f32)
                        if s < NS - 1:
                            kgdbT = p_t.tile([128, 2 * C], F32R, tag="kgdbT",
                                             name=f"kgT{s}_{grp}_{half}", bufs=1)
                            nc.scalar.activation(kgdbT[:], tp[:], AF.Copy)
                            dbv = p_s.tile([128, 2], F32, tag="dbv",
                                           name=f"dbv{s}_{grp}_{half}", bufs=4)
                            nc.vector.tensor_scalar_mul(
                                dbv[:], ebsl.bitcast(F32)[:, C - 1:2 * C:C],
                                1.0 / SCALE)
                            for u in range(2):
                                kt = kt0 + u
                                upd = ps_upd.tile([128, D], F32, tag="upd",
                                                  name=f"up{s}_{kt}")
                                for j in range(CC):
                                    nc.tensor.matmul(
                                        upd[:],
                                        kgdbT[:, (2 * u + j) * 128:
                                              (2 * u + j + 1) * 128],
                                        vtile(s, j), start=(j == 0),
                                        stop=(s == 0 and j == CC - 1))
                                if s > 0:
                                    nc.tensor.matmul(upd[:], id_sb[:],
                                                     h_tiles[kt][:],
                                                     start=False, stop=True)
                                nc.scalar.activation(h_tiles[kt][:], upd[:],
                                                     AF.Copy,
                                                     scale=dbv[:, u:u + 1])

                at_sb = []
                for j in range(CC):
                    m = p_t.tile([128, C], F32R, tag=f"atsb{j}",
                                 name=f"atsb{j}_{s}")
                    nc.vector.tensor_tensor(
                        m[:], at_ps[j][:],
                        triu_sb[:, j * C:(j + 1) * C].bitcast(F32), ALU.mult)
                    at_sb.append(m)
                for g in range(CC):
                    for j in range(CC):
                        nc.tensor.matmul(o_ps[g][:],
                                         at_sb[j][:, g * 128:(g + 1) * 128],
                                         vtile(s, j),
                                         start=(s == 0 and j == 0),
                                         stop=(j == CC - 1))

                ykvt = p_s.tile([128, CC * C], F32R, tag="ykvt", name=f"ykvt{s}", bufs=1)
                for g in range(CC):
                    o_t = o_ps[g]
                    s1 = p_s.tile([128, 1], F32, tag="s1", name=f"s1_{s}_{g}")
                    nc.vector.tensor_reduce(s1[:], o_t[:], mybir.AxisListType.X,
                                            ALU.add)
                    sq = p_t.tile([128, D], F32, tag="sqscr", name=f"sq{s}_{g}")
                    s2 = p_s.tile([128, 1], F32, tag="s2", name=f"s2_{s}_{g}")
                    nc.scalar.activation(sq[:], o_t[:], AF.Square, accum_out=s2[:])
                    mean = p_s.tile([128, 1], F32, tag="mean", name=f"mn{s}_{g}")
                    nc.vector.tensor_scalar_mul(mean[:], s1[:], 1.0 / D)
                    var = p_s.tile([128, 1], F32, tag="var", name=f"vr{s}_{g}")
                    nc.vector.tensor_scalar_mul(var[:], s2[:], 1.0 / D)
                    msq = p_s.tile([128, 1], F32, tag="msq", name=f"mq{s}_{g}")
                    nc.vector.tensor_tensor(msq[:], mean[:], mean[:], ALU.mult)
                    nc.vector.tensor_tensor(var[:], var[:], msq[:], ALU.subtract)
                    std = p_s.tile([128, 1], F32, tag="std", name=f"sd{s}_{g}")
                    nc.scalar.activation(std[:], var[:], AF.Sqrt, bias=eps_c[:])
                    rstd = p_s.tile([128, 1], F32, tag="rstd", name=f"rs{s}_{g}")
                    nc.vector.reciprocal(rstd[:], std[:])
                    nbias = p_s.tile([128, 1], F32, tag="nbias", name=f"nb{s}_{g}")
                    nc.vector.tensor_tensor(nbias[:], mean[:], rstd[:], ALU.mult)
                    nc.vector.tensor_scalar_mul(nbias[:], nbias[:], -1.0)
                    ykv = p_t.tile([128, D], F32, tag="ykv", name=f"ykv{s}_{g}")
                    nc.scalar.activation(ykv[:], o_t[:], AF.Identity,
                                         scale=rstd[:], bias=nbias[:])
                    tp2 = ps_upd.tile([128, C], F32, tag="tp", name=f"tpy{s}_{g}")
                    for d_ in range(DT):
                        nc.tensor.transpose(tp2[:, d_ * 128:(d_ + 1) * 128],
                                            ykv[:, d_ * 128:(d_ + 1) * 128], id_f32)
                    for d_ in range(DT):
                        nc.scalar.activation(
                            ykvt[:, d_ * C + g * 128: d_ * C + (g + 1) * 128],
                            tp2[:, d_ * 128:(d_ + 1) * 128], AF.Copy)

                dec_ps = [ps_at.tile([128, D], F32, tag=f"at{g}",
                                     name=f"dc{g}_{s}") for g in range(CC)]
                for kp in range(KT // 2):     # kt pairs
                    kt0 = 2 * kp
                    if kt0 % 4 == 0:
                        ks = kt0 // 4
                        wvsl = p_w.tile([128, 4 * DT * 128], F32R, tag="wencv",
                                        name=f"wvs{s}_{ks}")
                        nc.sync.dma_start(wvsl[:], wencv[ks])
                        wdsl = p_wd.tile([128, 4 * D], F32R, tag="wdec",
                                         name=f"wds{s}_{ks}")
                        nc.sync.dma_start(wdsl[:], wdec[ks])
                    evps = ps_upd.tile([128, 2 * C], F32,
                                       tag=("tp" if kp % 2 == 0 else "upd"),
                                       name=f"pv{s}_{kp}")
                    for u in range(2):
                        ki = (kt0 + u) % 4
                        for d_ in range(DT):
                            nc.tensor.matmul(
                                evps[:, u * C:(u + 1) * C],
                                wvsl[:, ki * 256 + d_ * 128:
                                     ki * 256 + d_ * 128 + 128],
                                ykvt[:, d_ * C:(d_ + 1) * C],
                                start=(d_ == 0), stop=(d_ == DT - 1))
                    t1 = p_t.tile([128, 2 * C], F32, tag="t1",
                                  name=f"t1_{s}_{kp}", bufs=1)
                    xsp_pair_ap = xsp_tiles[kt0].tensor.ap()[:, 0:2 * C]
                    nc.vector.scalar_tensor_tensor(
                        t1[:], evps[:], 0.0, xsp_pair_ap, ALU.max, ALU.mult)
                    xy = p_t.tile([128, 2 * C], F32R, tag="xy",
                                  name=f"xy{s}_{kp}")
                    nc.vector.scalar_tensor_tensor(
                        xy[:], evps[:], 0.0, t1[:], ALU.bypass, ALU.mult)
                    for u in range(2):
                        kt = kt0 + u
                        ki = kt % 4
                        for g in range(CC):
                            nc.tensor.matmul(
                                dec_ps[g][:],
                                xy[:, u * C + g * 128: u * C + (g + 1) * 128],
                                wdsl[:, ki * D:(ki + 1) * D],
                                start=(kt == 0), stop=(kt == KT - 1))
                for g in range(CC):
                    t_ = s * CC + g
                    nc.scalar.activation(yMLP[:, t_ * D:(t_ + 1) * D],
                                         dec_ps[g][:], AF.Copy)
                t0 = s * CC
                if N_CORES > 1:
                    for g in range(CC):
                        t_ = t0 + g
                        nc.sync.dma_start(ar_in[t_],
                                          yMLP[:, t_ * D:(t_ + 1) * D])
                    nc.gpsimd.collective_compute(
                        "AllReduce", ALU.add, replica_groups=GROUPS,
                        ins=[ar_in[t0:t0 + CC]], outs=[ar_out[t0:t0 + CC]])
                    for g in range(CC):
                        t_ = t0 + g
                        nc.sync.dma_start(yMLP[:, t_ * D:(t_ + 1) * D],
                                          ar_out[t_])


            for t_ in range(TT):
                final_tile(t_)

    nc.compile()
    return nc


def _tile_w(W):
    # (D, NK) -> (KT//4, 128, 4*DT*128): slab ks holds k-tiles 4ks..4ks+3,
    # column order (ki, d, c) matching the SBUF slab layout.
    W = np.asarray(W, dtype=np.float32)
    a = W.reshape(DT, 128, KT // 4, 4, 128).transpose(2, 1, 3, 0, 4)
    return np.ascontiguousarray(a.reshape(KT // 4, 128, 4 * DT * 128))


def _tile_wdec(W):
    # (NK, D) -> (KT//4, 128, 4*D)
    W = np.asarray(W, dtype=np.float32)
    a = W.reshape(KT // 4, 4, 128, D).transpose(0, 2, 1, 3)
    return np.ascontiguousarray(a.reshape(KT // 4, 128, 4 * D))


_STATE = {}


def _get_nc():
    if "nc" not in _STATE:
        _STATE["nc"] = _build()
    return _STATE["nc"]


def _core_in_map(x_b, W_enc_h, W_gate_h, W_encv_h, W_dec_h, consts):
    cos_t, sin_t, triu, ident = consts
    xT = np.ascontiguousarray(x_b.T)
    return {
        "wenc": _tile_w(W_enc_h),
        "wgate": _tile_w(W_gate_h),
        "wencv": _tile_w(W_encv_h),
        "wdec": _tile_wdec(W_dec_h),
        "xt": np.ascontiguousarray(xT.reshape(DT, 128, T)),
        "xv": np.ascontiguousarray(x_b.reshape(TT, 128, D)),
        "cos_t": cos_t, "sin_t": sin_t, "triu": triu, "ident": ident,
    }


def make_in_maps(x, W_enc, W_gate, W_dec, W_encv):
    x = np.asarray(x, dtype=np.float32)
    W_enc = np.asarray(W_enc, dtype=np.float32)
    W_gate = np.asarray(W_gate, dtype=np.float32)
    W_dec = np.asarray(W_dec, dtype=np.float32)
    W_encv = np.asarray(W_encv, dtype=np.float32)

    cos_t, sin_t = _rope_tables()
    triu = np.zeros((CC, 128, C), dtype=np.float32)
    for j in range(CC):
        for p in range(128):
            triu[j, p, j * 128 + p:] = 1.0
    ident = np.eye(128, dtype=np.float32)
    consts = (cos_t, sin_t, triu, ident)

    in_maps = []
    for c in range(N_CORES):
        b, h = c // 4, c % 4
        nsl = slice(h * N, (h + 1) * N)
        in_maps.append(_core_in_map(
            x[b], W_enc[:, nsl], W_gate[:, nsl], W_encv[h], W_dec[nsl, :],
            consts))
    return in_maps


def _get_runner():
    """Cached jitted SPMD executable mirroring bass2jax.run_bass_via_pjrt, so
    repeated kernel() calls skip re-tracing/recompiling."""
    if "runner" in _STATE:
        return _STATE["runner"]
    import jax
    import concourse.mybir as mb
    from concourse import bass2jax as b2j
    from jax.experimental.shard_map import shard_map
    from jax.sharding import Mesh, PartitionSpec

    nc = _get_nc()
    b2j.install_neuronx_cc_hook()
    partition_name = (nc.partition_id_tensor.name
                      if nc.partition_id_tensor else None)
    in_names, out_names, out_avals, zero_outs = [], [], [], []
    for alloc in nc.m.functions[0].allocations:
        if not isinstance(alloc, mb.MemoryLocationSet):
            continue
        name = alloc.memorylocations[0].name
        if alloc.kind == "ExternalInput":
            if name != partition_name:
                in_names.append(name)
        elif alloc.kind == "ExternalOutput":
            shape = tuple(alloc.tensor_shape)
            dtype = mb.dt.np(alloc.dtype)
            out_names.append(name)
            out_avals.append(jax.core.ShapedArray(shape, dtype))
            zero_outs.append(np.zeros(shape, dtype))
    n_params = len(in_names)
    all_names = in_names + out_names
    if partition_name is not None:
        all_names = all_names + [partition_name]
    donate = tuple(range(n_params, n_params + len(out_names)))

    def _body(*args):
        operands = list(args)
        if partition_name is not None:
            operands.append(b2j.partition_id_tensor())
        return tuple(b2j._bass_exec_p.bind(
            *operands,
            out_avals=tuple(out_avals),
            in_names=tuple(all_names),
            out_names=tuple(out_names),
            lowering_input_output_aliases=(),
            sim_require_finite=True,
            sim_require_nnan=True,
            nc=nc,
        ))

    devices = jax.devices()[:N_CORES]
    mesh = Mesh(np.asarray(devices), ("core",))
    in_specs = (PartitionSpec("core"),) * (n_params + len(out_names))
    out_specs = (PartitionSpec("core"),) * len(out_names)
    sharded = jax.jit(
        shard_map(_body, mesh=mesh, in_specs=in_specs, out_specs=out_specs,
                  check_rep=False),
        donate_argnums=donate, keep_unused=True)
    _STATE["runner"] = (sharded, in_names, out_names, out_avals, zero_outs, mesh)
    return _STATE["runner"]


def _concat_inputs(in_maps, in_names):
    return [np.concatenate([np.asarray(in_maps[c][nm]) for c in range(N_CORES)],
                           axis=0) for nm in in_names]


def _run(in_maps):
    sharded, in_names, out_names, out_avals, zero_outs, mesh = _get_runner()
    concat_in = _concat_inputs(in_maps, in_names)
    concat_zeros = [np.zeros((N_CORES * z.shape[0], *z.shape[1:]), z.dtype)
                    for z in zero_outs]
    out_arrs = sharded(*concat_in, *concat_zeros)
    return {name: np.asarray(out_arrs[i]).reshape(N_CORES, *out_avals[i].shape)
            for i, name in enumerate(out_names)}


def kernel(x, W_enc, W_gate, W_dec, W_encv):
    in_maps = make_in_maps(x, W_enc, W_gate, W_dec, W_encv)
    outs = _run(in_maps)
    y0 = outs["y"][0].reshape(T, D)
    y1 = outs["y"][4].reshape(T, D)
    return np.stack([y0, y1]).astype(np.float32)


def time_device_exec(np_inputs, iters=10):
    """Best wall-clock (ns) of the device execution with inputs pre-staged on
    device; excludes host prep and output conversion."""
    import time as _time
    import jax
    from jax.sharding import NamedSharding, PartitionSpec
    sharded, in_names, out_names, out_avals, zero_outs, mesh = _get_runner()
    in_maps = make_in_maps(**np_inputs)
    concat_in = _concat_inputs(in_maps, in_names)
    sh = NamedSharding(mesh, PartitionSpec("core"))
    dev_in = [jax.device_put(a, sh) for a in concat_in]
    for a in dev_in:
        a.block_until_ready()
    best = float("inf")
    for _ in range(iters):
        concat_zeros = [jax.device_put(
            np.zeros((N_CORES * z.shape[0], *z.shape[1:]), z.dtype), sh)
            for z in zero_outs]
        for a in concat_zeros:
            a.block_until_ready()
        t0 = _time.perf_counter()
        out = sharded(*dev_in, *concat_zeros)
        for o in out:
            o.block_until_ready()
        best = min(best, _time.perf_counter() - t0)
    return best * 1e9
